# revision 3
# baseline (speedup 1.0000x reference)
"""Trainium2 Bass kernel for nn_CGPCoupler (sparse Clebsch-Gordan bilinear coupling).

Reference computation:
    out[:, ro] += x1[:, r1] * x2[:, r2] * cg        (nnz = 9856 sparse entries)

Structure exploited: the index triples come in 16-wide aligned runs, so the whole
op factors over 16-element "subslots" (40 of them in the 640-dim rep space):

    out_O  +=  c_t * (x1_A  (*)  x2_B)      for 616 subslot-triples t=(A,B,O,c)

with only D=308 distinct (A,B) products (provably minimal: every (l1,l2) family
couples to all its allowed lout, so the joint coupling tensor has full product
rank). Dataflow (per core, data parallel over the batch dim, 1024 rows/core,
fp16 datapath / fp32 PSUM):

    layout:  x2f[p = subslot*2 + ch_half (80 partitions), f = n*8 + ch_lo (8192)]
    host:    x1g = x1 replicated into product-row order (numpy fancy-index),
             streamed straight from HBM (no on-chip gather for side 1)
    1. G2 = SEL2^T @ x2f      (TensorE one-hot selection matmul -> PSUM)
    2. P  = x1g * G2          (VectorE 2x fp16 after ScalarE evacuates some
                               chunks; VectorE multiplies the rest straight
                               out of PSUM at 1x)
    3. out = W^T @ P          (TensorE, CG coeffs folded into constant fp16 W,
                               PSUM-accumulated over the 5 product-row chunks)

v5 pipeline (vs v3 baseline at 62.3us):
  - stationary reuse: gather/scatter loop chunks OUTER over 2-super blocks, so
    each SEL2/W chunk is loaded once per 4 matmuls (was 1:1) -> ~4x fewer
    LDWEIGHTS on the PE pipe.
  - exact 8-bank PSUM plan: 4 rotating gather banks [128,512]f32 + 2x2-bank
    output accumulators [80,1024]f32 (two supers in flight).
  - measured-cost-balanced evacuation: ScalarE evacuates NEVAC of 10 segs
    (0.61us each) + VectorE multiplies them in 2x fp16 (0.33us); VectorE
    multiplies the rest directly from PSUM at 1x (0.66us).
  - piecewise x2f loads (per-super 160KB) so the first gather starts ~0.6us in.
"""

import os
import sys
import types

import numpy as np


def _ensure_ntff_hook():
    """concourse's trace path imports antenv.axon_hooks, which this image's
    antenv lacks. Provide it (and register the real profiling hook when the
    axon boot module is available) so tracing works instead of crashing."""
    try:
        import antenv
    except ImportError:
        return
    if getattr(antenv, "axon_hooks", None) is not None:
        return
    try:
        from antenv import axon_hooks  # noqa: F401
        return
    except ImportError:
        pass
    mod = types.ModuleType("antenv.axon_hooks")
    state = {"hook": None}
    mod.set_axon_ntff_profile_hook = lambda h: state.__setitem__("hook", h)
    mod.get_axon_ntff_profile_hook = lambda: state["hook"]
    sys.modules["antenv.axon_hooks"] = mod
    antenv.axon_hooks = mod
    try:
        from trn_agent_boot.trn_boot import _ntff_profile_via_ctypes
        so = "/opt/axon/libaxon_pjrt.so"
        if os.path.exists(so):
            mod.set_axon_ntff_profile_hook(_ntff_profile_via_ctypes(so))
    except Exception:
        pass


_ensure_ntff_hook()

N = 8192
DIM = 640
NCORES = 8
NLOC = N // NCORES          # rows per core
NSUB = DIM // 16            # 40 subslots
P_IN = NSUB * 2             # 80 partitions: (subslot, ch-half)
CHH = 8                     # channels per half
FTOT = NLOC * CHH           # 8192 free elements per partition
FSUP = 1024                 # free-dim super-chunk (per DMA / out tile)
FSEG = 512                  # free-dim segment per matmul (one PSUM bank, fp32)
NSUP = FTOT // FSUP         # 8 supers
NSEG = FSUP // FSEG         # 2 segments per super
SBLK = 2                    # supers per stationary block

LAST_RESULTS = None         # BassKernelResults of the most recent run

_matrices_cache = {}
_program_cache = {}


def _build_matrices(cg, r1, r2, ro):
    """Derive subslot terms from the sparse index lists and build the constant
    SEL2/W matrices. Everything is validated with asserts."""
    key = (r1.tobytes(), r2.tobytes(), ro.tobytes(), cg.tobytes())
    hit = _matrices_cache.get(key)
    if hit is not None:
        return hit

    A = r1 // 16
    B = r2 // 16
    O = ro // 16
    j = r1 % 16
    assert (r2 % 16 == j).all() and (ro % 16 == j).all(), \
        "index triples are not 16-aligned runs"
    assert A.max() < NSUB and B.max() < NSUB and O.max() < NSUB

    terms = {}   # (A,B,O) -> [coeff, covered-bitmask]
    for a, b, o, jj, c in zip(A.tolist(), B.tolist(), O.tolist(),
                              j.tolist(), cg.tolist()):
        k = (a, b, o)
        e = terms.get(k)
        if e is None:
            terms[k] = [c, 1 << jj]
        else:
            assert e[0] == c, "coefficient varies within a 16-run"
            assert not (e[1] >> jj) & 1, "duplicate (A,B,O,j) entry"
            e[1] |= 1 << jj
    for k, (c, mask) in terms.items():
        assert mask == 0xFFFF, f"term {k} covers only mask {mask:#x}"

    products = sorted({(a, b) for (a, b, o) in terms})
    pidx = {ab: d for d, ab in enumerate(products)}
    D = len(products)
    D2 = 2 * D
    nchunks = (D2 + 127) // 128
    D2p = 128 * nchunks

    SEL2 = np.zeros((P_IN, D2p), np.float16)
    A2 = np.zeros(D2p, np.int64)   # product row -> source row in x1f layout
    W = np.zeros((D2p, P_IN), np.float16)
    for (a, b), d in pidx.items():
        for hh in (0, 1):
            SEL2[b * 2 + hh, 2 * d + hh] = 1.0
            A2[2 * d + hh] = a * 2 + hh
    for (a, b, o), (c, _) in terms.items():
        d = pidx[(a, b)]
        for hh in (0, 1):
            W[2 * d + hh, o * 2 + hh] = c

    # pack W row-chunks side by side: WPACK[:, c*P_IN:(c+1)*P_IN] = W[c*128:...]
    WPACK = np.zeros((128, nchunks * P_IN), np.float16)
    for c in range(nchunks):
        WPACK[:, c * P_IN:(c + 1) * P_IN] = W[c * 128:(c + 1) * 128, :]

    out = (A2, SEL2, WPACK, nchunks)
    _matrices_cache[key] = out
    return out


def _pack_x(x):
    """[NLOC, 640] -> [80, NLOC*8] fp16: row p = subslot*2 + half, col = n*8 + ch."""
    return np.ascontiguousarray(
        x.reshape(NLOC, NSUB, 2, CHH).transpose(1, 2, 0, 3).reshape(P_IN, FTOT),
        dtype=np.float16)


def _unpack_out(o):
    """[80, NLOC*8] -> [NLOC, 640]."""
    return o.reshape(NSUB, 2, NLOC, CHH).transpose(2, 0, 1, 3).reshape(NLOC, DIM)


# ---- tuning knobs -----------------------------------------------------------
NEVAC = 6      # of the 10 gather segments per super, how many ScalarE evacuates
               # (VectorE handles the rest straight from PSUM at 1x)


def _build_program(nchunks):
    import concourse.mybir as mybir
    import concourse.tile as tile
    from concourse import bacc
    from concourse.bass import ds, ts

    f32 = mybir.dt.float32
    f16 = mybir.dt.float16
    nc = bacc.Bacc("TRN2", target_bir_lowering=False)

    x1gd = nc.dram_tensor("x1g", [nchunks, 128, FTOT], f16, kind="ExternalInput")
    x2d = nc.dram_tensor("x2f", [P_IN, FTOT], f16, kind="ExternalInput")
    s2d = nc.dram_tensor("sel2", [P_IN, nchunks * 128], f16, kind="ExternalInput")
    wd = nc.dram_tensor("wmat", [128, nchunks * P_IN], f16, kind="ExternalInput")
    outd = nc.dram_tensor("outf", [P_IN, FTOT], f16, kind="ExternalOutput")

    NBLK = NSUP // SBLK

    with tile.TileContext(nc) as tc:
        with tc.tile_pool(name="const", bufs=1) as constp, \
             tc.tile_pool(name="x1io", bufs=3 * nchunks) as x1io, \
             tc.tile_pool(name="x2io", bufs=4) as x2io, \
             tc.tile_pool(name="gsb", bufs=10) as gsb, \
             tc.tile_pool(name="psb", bufs=3 * SBLK * NSEG * nchunks) as psb, \
             tc.tile_pool(name="og", bufs=4) as og, \
             tc.tile_pool(name="psg", bufs=4, space="PSUM") as psg, \
             tc.tile_pool(name="pso", bufs=2, space="PSUM") as pso:

            s2 = constp.tile([P_IN, nchunks * 128], f16, name="s2", tag="s2")
            nc.scalar.dma_start(out=s2, in_=s2d[:])
            w = constp.tile([128, nchunks * P_IN], f16, name="w", tag="w")
            nc.scalar.dma_start(out=w, in_=wd[:])

            for blk in range(NBLK):
                sups = [blk * SBLK + i for i in range(SBLK)]

                # ---- input DMAs for this block -----------------------------
                x2t = {}
                x1gt = {}
                for s in sups:
                    ssl = ds(s * FSUP, FSUP)
                    t = x2io.tile([P_IN, FSUP], f16, name="x2t", tag="x2t")
                    nc.scalar.dma_start(out=t, in_=x2d[:, ssl])
                    x2t[s] = t
                    for c in range(nchunks):
                        g = x1io.tile([128, FSUP], f16, name="x1g", tag="x1g")
                        nc.sync.dma_start(
                            out=g, in_=x1gd[c, :, s * FSUP:(s + 1) * FSUP])
                        x1gt[s, c] = g

                # ---- gather + multiply: chunk-outer for stationary reuse ---
                # Per (c): SEL2[c] stays stationary for SBLK*NSEG matmuls.
                pt = {}
                evac_budget = NEVAC * SBLK  # per block, out of 10*SBLK segs
                ei = 0
                for c in range(nchunks):
                    for s in sups:
                        g2p = psg.tile([128, FSEG], f32, name="gp", tag="gp")
                        g2p2 = psg.tile([128, FSEG], f32, name="gp2", tag="gp")
                        segs = (g2p, g2p2)
                        for jseg in range(NSEG):
                            nc.tensor.matmul(
                                segs[jseg], s2[:, ts(c, 128)],
                                x2t[s][:, ts(jseg, FSEG)],
                                start=True, stop=True)
                        for jseg in range(NSEG):
                            p = psb.tile([128, FSEG], f16, name="pt", tag="pt")
                            x1seg = x1gt[s, c][:, ts(jseg, FSEG)]
                            if ei < evac_budget:
                                # ScalarE evacuates; VectorE multiplies 2x fp16
                                g2s = gsb.tile([128, FSEG], f16, name="g2s", tag="g2s")
                                nc.scalar.copy(out=g2s, in_=segs[jseg])
                                nc.vector.tensor_mul(p, x1seg, g2s)
                            else:
                                # VectorE multiplies straight from PSUM (1x)
                                nc.vector.tensor_mul(p, x1seg, segs[jseg])
                            ei += 1
                            pt[s, c, jseg] = p

                # ---- scatter: chunk-outer, accumulate into per-super banks -
                outp = {}
                for s in sups:
                    outp[s] = pso.tile([P_IN, FSUP], f32, name="outp", tag="outp")
                for c in range(nchunks):
                    for s in sups:
                        for jseg in range(NSEG):
                            nc.tensor.matmul(
                                outp[s][:, ts(jseg, FSEG)],
                                w[:, ts(c, P_IN)],
                                pt[s, c, jseg],
                                start=(c == 0), stop=(c == nchunks - 1),
                                skip_group_check=True)

                # ---- cast + store ------------------------------------------
                last = (blk == NBLK - 1)
                for i, s in enumerate(sups):
                    outt = og.tile([P_IN, FSUP], f16, name="outt", tag="outt")
                    # split the two segment casts across VectorE / ScalarE
                    nc.vector.tensor_copy(out=outt[:, ts(0, FSEG)],
                                          in_=outp[s][:, ts(0, FSEG)])
                    nc.scalar.copy(out=outt[:, ts(1, FSEG)],
                                   in_=outp[s][:, ts(1, FSEG)])
                    ssl = ds(s * FSUP, FSUP)
                    if last and i == SBLK - 1:
                        # kernel tail: low-latency HWDGE path
                        nc.scalar.dma_start(out=outd[:, ssl], in_=outt)
                    else:
                        nc.gpsimd.dma_start(out=outd[:, ssl], in_=outt)
    nc.compile()
    return nc


def kernel(x1, x2, cg_tilde, repids_in1, repids_in2, repids_out, out_dim=DIM,
           **_ignored):
    global LAST_RESULTS
    import concourse.bass_utils as _bu
    from concourse.bass_utils import run_bass_kernel_spmd
    # the trace path uploads artifacts to S3, which this container can't reach
    if not getattr(_bu.upload_artifacts, "_local", False):
        _bu.upload_artifacts = lambda tmpdir: "local://" + tmpdir
        _bu.upload_artifacts._local = True

    x1 = np.ascontiguousarray(np.asarray(x1), dtype=np.float32)
    x2 = np.ascontiguousarray(np.asarray(x2), dtype=np.float32)
    cg = np.asarray(cg_tilde, dtype=np.float32)
    r1 = np.asarray(repids_in1, dtype=np.int64)
    r2 = np.asarray(repids_in2, dtype=np.int64)
    ro = np.asarray(repids_out, dtype=np.int64)
    out_dim = int(out_dim)
    assert x1.shape == (N, DIM) and x2.shape == (N, DIM) and out_dim == DIM

    A2, SEL2, WPACK, nchunks = _build_matrices(cg, r1, r2, ro)

    nc = _program_cache.get(nchunks)
    if nc is None:
        nc = _build_program(nchunks)
        _program_cache[nchunks] = nc

    in_maps = []
    for c in range(NCORES):
        sl = slice(c * NLOC, (c + 1) * NLOC)
        x1f = _pack_x(x1[sl])
        in_maps.append({
            "x1g": np.ascontiguousarray(
                x1f[A2].reshape(nchunks, 128, FTOT)),
            "x2f": _pack_x(x2[sl]),
            "sel2": SEL2,
            "wmat": WPACK,
        })

    res = run_bass_kernel_spmd(nc, in_maps, core_ids=list(range(NCORES)))
    LAST_RESULTS = res

    out = np.empty((N, DIM), np.float32)
    for c in range(NCORES):
        out[c * NLOC:(c + 1) * NLOC] = _unpack_out(
            np.asarray(res.results[c]["outf"], dtype=np.float32))
    return out


def _numpy_model(x1, x2, cg, r1, r2, ro):
    """Host-side model of the device dataflow (including fp16 quantization),
    for validating index logic and predicting the on-device error."""
    A2, SEL2, WPACK, nchunks = _build_matrices(cg, r1, r2, ro)
    W = np.zeros((128 * nchunks, P_IN), np.float32)
    for c in range(nchunks):
        W[c * 128:(c + 1) * 128, :] = WPACK[:, c * P_IN:(c + 1) * P_IN].astype(
            np.float32)
    out = np.empty_like(x1)
    for c in range(NCORES):
        sl = slice(c * NLOC, (c + 1) * NLOC)
        x1f = _pack_x(x1[sl])
        x2f = _pack_x(x2[sl]).astype(np.float32)
        g1 = x1f[A2].astype(np.float32)
        g2 = (SEL2.astype(np.float32).T @ x2f).astype(np.float16)  # worst branch
        p = (g1 * g2.astype(np.float32)).astype(np.float16)
        outf = W.T @ p.astype(np.float32)
        out[sl] = _unpack_out(outf)
    return out


# revision 5
# speedup vs baseline: 1.1511x; 1.1511x over previous
"""Trainium2 Bass kernel for nn_CGPCoupler (sparse Clebsch-Gordan bilinear coupling).

Reference computation:
    out[:, ro] += x1[:, r1] * x2[:, r2] * cg        (nnz = 9856 sparse entries)

Structure exploited: the index triples come in 16-wide aligned runs, so the whole
op factors over 16-element "subslots" (40 of them in the 640-dim rep space):

    out_O  +=  c_t * (x1_A  (*)  x2_B)      for 616 subslot-triples t=(A,B,O,c)

with only D=308 distinct (A,B) products (provably minimal: every (l1,l2) family
couples to all its allowed lout, so the joint coupling tensor has full product
rank). Dataflow (per core, data parallel over the batch dim, 1024 rows/core,
fp16 datapath / fp32 PSUM):

    layout:  x2f[p = subslot*2 + ch_half (80 partitions), f = n*8 + ch_lo (8192)]
    host:    x1g = x1 replicated into product-row order (numpy fancy-index),
             streamed straight from HBM (no on-chip gather for side 1)
    1. G2 = SEL2^T @ x2f      (TensorE one-hot selection matmul -> PSUM)
    2. P  = x1g * G2          (VectorE 2x fp16 after ScalarE evacuates some
                               chunks; VectorE multiplies the rest straight
                               out of PSUM at 1x)
    3. out = W^T @ P          (TensorE, CG coeffs folded into constant fp16 W,
                               PSUM-accumulated over the 5 product-row chunks)

v5 pipeline (vs v3 baseline at 62.3us):
  - stationary reuse: gather/scatter loop chunks OUTER over 2-super blocks, so
    each SEL2/W chunk is loaded once per 4 matmuls (was 1:1) -> ~4x fewer
    LDWEIGHTS on the PE pipe.
  - exact 8-bank PSUM plan: 4 rotating gather banks [128,512]f32 + 2x2-bank
    output accumulators [80,1024]f32 (two supers in flight).
  - measured-cost-balanced evacuation: ScalarE evacuates NEVAC of 10 segs
    (0.61us each) + VectorE multiplies them in 2x fp16 (0.33us); VectorE
    multiplies the rest directly from PSUM at 1x (0.66us).
  - piecewise x2f loads (per-super 160KB) so the first gather starts ~0.6us in.
"""

import os
import sys
import types

import numpy as np


def _ensure_ntff_hook():
    """concourse's trace path imports antenv.axon_hooks, which this image's
    antenv lacks. Provide it (and register the real profiling hook when the
    axon boot module is available) so tracing works instead of crashing."""
    try:
        import antenv
    except ImportError:
        return
    if getattr(antenv, "axon_hooks", None) is not None:
        return
    try:
        from antenv import axon_hooks  # noqa: F401
        return
    except ImportError:
        pass
    mod = types.ModuleType("antenv.axon_hooks")
    state = {"hook": None}
    mod.set_axon_ntff_profile_hook = lambda h: state.__setitem__("hook", h)
    mod.get_axon_ntff_profile_hook = lambda: state["hook"]
    sys.modules["antenv.axon_hooks"] = mod
    antenv.axon_hooks = mod
    try:
        from trn_agent_boot.trn_boot import _ntff_profile_via_ctypes
        so = "/opt/axon/libaxon_pjrt.so"
        if os.path.exists(so):
            mod.set_axon_ntff_profile_hook(_ntff_profile_via_ctypes(so))
    except Exception:
        pass


_ensure_ntff_hook()

N = 8192
DIM = 640
NCORES = 8
NLOC = N // NCORES          # rows per core
NSUB = DIM // 16            # 40 subslots
P_IN = NSUB * 2             # 80 partitions: (subslot, ch-half)
CHH = 8                     # channels per half
FTOT = NLOC * CHH           # 8192 free elements per partition
FSUP = 1024                 # free-dim super-chunk (per DMA / out tile)
FSEG = 512                  # free-dim segment per matmul (one PSUM bank, fp32)
NSUP = FTOT // FSUP         # 8 supers
NSEG = FSUP // FSEG         # 2 segments per super
SBLK = 2                    # supers per stationary block

LAST_RESULTS = None         # BassKernelResults of the most recent run

_matrices_cache = {}
_program_cache = {}


def _build_matrices(cg, r1, r2, ro):
    """Derive subslot terms from the sparse index lists and build the constant
    SEL2/W matrices. Everything is validated with asserts."""
    key = (r1.tobytes(), r2.tobytes(), ro.tobytes(), cg.tobytes())
    hit = _matrices_cache.get(key)
    if hit is not None:
        return hit

    A = r1 // 16
    B = r2 // 16
    O = ro // 16
    j = r1 % 16
    assert (r2 % 16 == j).all() and (ro % 16 == j).all(), \
        "index triples are not 16-aligned runs"
    assert A.max() < NSUB and B.max() < NSUB and O.max() < NSUB

    terms = {}   # (A,B,O) -> [coeff, covered-bitmask]
    for a, b, o, jj, c in zip(A.tolist(), B.tolist(), O.tolist(),
                              j.tolist(), cg.tolist()):
        k = (a, b, o)
        e = terms.get(k)
        if e is None:
            terms[k] = [c, 1 << jj]
        else:
            assert e[0] == c, "coefficient varies within a 16-run"
            assert not (e[1] >> jj) & 1, "duplicate (A,B,O,j) entry"
            e[1] |= 1 << jj
    for k, (c, mask) in terms.items():
        assert mask == 0xFFFF, f"term {k} covers only mask {mask:#x}"

    products = sorted({(a, b) for (a, b, o) in terms})
    pidx = {ab: d for d, ab in enumerate(products)}
    D = len(products)
    D2 = 2 * D
    nchunks = (D2 + 127) // 128
    D2p = 128 * nchunks

    SEL2 = np.zeros((P_IN, D2p), np.float16)
    A2 = np.zeros(D2p, np.int64)   # product row -> source row in x1f layout
    W = np.zeros((D2p, P_IN), np.float16)
    for (a, b), d in pidx.items():
        for hh in (0, 1):
            SEL2[b * 2 + hh, 2 * d + hh] = 1.0
            A2[2 * d + hh] = a * 2 + hh
    for (a, b, o), (c, _) in terms.items():
        d = pidx[(a, b)]
        for hh in (0, 1):
            W[2 * d + hh, o * 2 + hh] = c

    # pack W row-chunks side by side: WPACK[:, c*P_IN:(c+1)*P_IN] = W[c*128:...]
    WPACK = np.zeros((128, nchunks * P_IN), np.float16)
    for c in range(nchunks):
        WPACK[:, c * P_IN:(c + 1) * P_IN] = W[c * 128:(c + 1) * 128, :]

    out = (A2, SEL2, WPACK, nchunks)
    _matrices_cache[key] = out
    return out


def _pack_x(x):
    """[NLOC, 640] -> [80, NLOC*8] fp16: row p = subslot*2 + half, col = n*8 + ch."""
    return np.ascontiguousarray(
        x.reshape(NLOC, NSUB, 2, CHH).transpose(1, 2, 0, 3).reshape(P_IN, FTOT),
        dtype=np.float16)


def _unpack_out(o):
    """[80, NLOC*8] -> [NLOC, 640]."""
    return o.reshape(NSUB, 2, NLOC, CHH).transpose(2, 0, 1, 3).reshape(NLOC, DIM)


# ---- tuning knobs -----------------------------------------------------------
NEVAC = 6      # of the 10 gather segments per super, how many ScalarE evacuates
               # (VectorE handles the rest straight from PSUM at 1x)


def _build_program(nchunks):
    import concourse.mybir as mybir
    import concourse.tile as tile
    from concourse import bacc
    from concourse.bass import ds, ts

    f32 = mybir.dt.float32
    f16 = mybir.dt.float16
    nc = bacc.Bacc("TRN2", target_bir_lowering=False)

    x1gd = nc.dram_tensor("x1g", [nchunks, 128, FTOT], f16, kind="ExternalInput")
    x2d = nc.dram_tensor("x2f", [P_IN, FTOT], f16, kind="ExternalInput")
    s2d = nc.dram_tensor("sel2", [P_IN, nchunks * 128], f16, kind="ExternalInput")
    wd = nc.dram_tensor("wmat", [128, nchunks * P_IN], f16, kind="ExternalInput")
    outd = nc.dram_tensor("outf", [P_IN, FTOT], f16, kind="ExternalOutput")

    NBLK = NSUP // SBLK

    with tile.TileContext(nc) as tc:
        with tc.tile_pool(name="const", bufs=1) as constp, \
             tc.tile_pool(name="x1io", bufs=2 * nchunks) as x1io, \
             tc.tile_pool(name="x2io", bufs=2) as x2io, \
             tc.tile_pool(name="gsb", bufs=10) as gsb, \
             tc.tile_pool(name="psb", bufs=3 * SBLK * NSEG * nchunks) as psb, \
             tc.tile_pool(name="og", bufs=4) as og, \
             tc.tile_pool(name="psg", bufs=4, space="PSUM") as psg, \
             tc.tile_pool(name="pso", bufs=2, space="PSUM") as pso:

            s2 = constp.tile([P_IN, nchunks * 128], f16, name="s2", tag="s2")
            nc.scalar.dma_start(out=s2, in_=s2d[:])
            w = constp.tile([128, nchunks * P_IN], f16, name="w", tag="w")
            nc.scalar.dma_start(out=w, in_=wd[:])

            for blk in range(NBLK):
                sups = [blk * SBLK + i for i in range(SBLK)]

                # ---- input DMAs for this block -----------------------------
                # x2 pieces ride the (otherwise idle) gpsimd SWDGE queue so
                # they never queue behind ScalarE evacuation work; x1g comes
                # in block-sized [128, 2048] transfers so the sync engine's
                # ~0.7us per-DMA issue cost stays well under the wire time.
                blk0 = blk * SBLK * FSUP
                x2b = x2io.tile([P_IN, SBLK * FSUP], f16, name="x2b", tag="x2b")
                nc.gpsimd.dma_start(out=x2b, in_=x2d[:, ds(blk0, SBLK * FSUP)])
                x2t = {s: x2b[:, ts(i, FSUP)] for i, s in enumerate(sups)}
                x1gb = {}
                for c in range(nchunks):
                    g = x1io.tile([128, SBLK * FSUP], f16, name="x1g", tag="x1g")
                    nc.sync.dma_start(
                        out=g, in_=x1gd[c, :, blk0:blk0 + SBLK * FSUP])
                    x1gb[c] = g
                x1gt = {(s, c): x1gb[c][:, ts(i, FSUP)]
                        for i, s in enumerate(sups) for c in range(nchunks)}

                # ---- gather + multiply: chunk-outer for stationary reuse ---
                # Per (c): SEL2[c] stays stationary for SBLK*NSEG matmuls.
                pt = {}
                evac_budget = NEVAC * SBLK  # per block, out of 10*SBLK segs
                ei = 0
                for c in range(nchunks):
                    for s in sups:
                        g2p = psg.tile([128, FSEG], f32, name="gp", tag="gp")
                        g2p2 = psg.tile([128, FSEG], f32, name="gp2", tag="gp")
                        segs = (g2p, g2p2)
                        for jseg in range(NSEG):
                            nc.tensor.matmul(
                                segs[jseg], s2[:, ts(c, 128)],
                                x2t[s][:, ts(jseg, FSEG)],
                                start=True, stop=True)
                        for jseg in range(NSEG):
                            p = psb.tile([128, FSEG], f16, name="pt", tag="pt")
                            x1seg = x1gt[s, c][:, ts(jseg, FSEG)]
                            if ei < evac_budget:
                                # ScalarE evacuates; VectorE multiplies 2x fp16
                                g2s = gsb.tile([128, FSEG], f16, name="g2s", tag="g2s")
                                nc.scalar.copy(out=g2s, in_=segs[jseg])
                                nc.vector.tensor_mul(p, x1seg, g2s)
                            else:
                                # VectorE multiplies straight from PSUM (1x)
                                nc.vector.tensor_mul(p, x1seg, segs[jseg])
                            ei += 1
                            pt[s, c, jseg] = p

                # ---- scatter: chunk-outer, accumulate into per-super banks -
                outp = {}
                for s in sups:
                    outp[s] = pso.tile([P_IN, FSUP], f32, name="outp", tag="outp")
                for c in range(nchunks):
                    for s in sups:
                        for jseg in range(NSEG):
                            nc.tensor.matmul(
                                outp[s][:, ts(jseg, FSEG)],
                                w[:, ts(c, P_IN)],
                                pt[s, c, jseg],
                                start=(c == 0), stop=(c == nchunks - 1),
                                skip_group_check=True)

                # ---- cast + store ------------------------------------------
                last = (blk == NBLK - 1)
                for i, s in enumerate(sups):
                    outt = og.tile([P_IN, FSUP], f16, name="outt", tag="outt")
                    # split the two segment casts across VectorE / ScalarE
                    nc.vector.tensor_copy(out=outt[:, ts(0, FSEG)],
                                          in_=outp[s][:, ts(0, FSEG)])
                    nc.scalar.copy(out=outt[:, ts(1, FSEG)],
                                   in_=outp[s][:, ts(1, FSEG)])
                    ssl = ds(s * FSUP, FSUP)
                    if last and i == SBLK - 1:
                        # kernel tail: low-latency HWDGE path
                        nc.scalar.dma_start(out=outd[:, ssl], in_=outt)
                    else:
                        nc.gpsimd.dma_start(out=outd[:, ssl], in_=outt)
    nc.compile()
    return nc


def kernel(x1, x2, cg_tilde, repids_in1, repids_in2, repids_out, out_dim=DIM,
           **_ignored):
    global LAST_RESULTS
    import concourse.bass_utils as _bu
    from concourse.bass_utils import run_bass_kernel_spmd
    # the trace path uploads artifacts to S3, which this container can't reach
    if not getattr(_bu.upload_artifacts, "_local", False):
        _bu.upload_artifacts = lambda tmpdir: "local://" + tmpdir
        _bu.upload_artifacts._local = True

    x1 = np.ascontiguousarray(np.asarray(x1), dtype=np.float32)
    x2 = np.ascontiguousarray(np.asarray(x2), dtype=np.float32)
    cg = np.asarray(cg_tilde, dtype=np.float32)
    r1 = np.asarray(repids_in1, dtype=np.int64)
    r2 = np.asarray(repids_in2, dtype=np.int64)
    ro = np.asarray(repids_out, dtype=np.int64)
    out_dim = int(out_dim)
    assert x1.shape == (N, DIM) and x2.shape == (N, DIM) and out_dim == DIM

    A2, SEL2, WPACK, nchunks = _build_matrices(cg, r1, r2, ro)

    nc = _program_cache.get(nchunks)
    if nc is None:
        nc = _build_program(nchunks)
        _program_cache[nchunks] = nc

    in_maps = []
    for c in range(NCORES):
        sl = slice(c * NLOC, (c + 1) * NLOC)
        x1f = _pack_x(x1[sl])
        in_maps.append({
            "x1g": np.ascontiguousarray(
                x1f[A2].reshape(nchunks, 128, FTOT)),
            "x2f": _pack_x(x2[sl]),
            "sel2": SEL2,
            "wmat": WPACK,
        })

    res = run_bass_kernel_spmd(nc, in_maps, core_ids=list(range(NCORES)))
    LAST_RESULTS = res

    out = np.empty((N, DIM), np.float32)
    for c in range(NCORES):
        out[c * NLOC:(c + 1) * NLOC] = _unpack_out(
            np.asarray(res.results[c]["outf"], dtype=np.float32))
    return out


def _numpy_model(x1, x2, cg, r1, r2, ro):
    """Host-side model of the device dataflow (including fp16 quantization),
    for validating index logic and predicting the on-device error."""
    A2, SEL2, WPACK, nchunks = _build_matrices(cg, r1, r2, ro)
    W = np.zeros((128 * nchunks, P_IN), np.float32)
    for c in range(nchunks):
        W[c * 128:(c + 1) * 128, :] = WPACK[:, c * P_IN:(c + 1) * P_IN].astype(
            np.float32)
    out = np.empty_like(x1)
    for c in range(NCORES):
        sl = slice(c * NLOC, (c + 1) * NLOC)
        x1f = _pack_x(x1[sl])
        x2f = _pack_x(x2[sl]).astype(np.float32)
        g1 = x1f[A2].astype(np.float32)
        g2 = (SEL2.astype(np.float32).T @ x2f).astype(np.float16)  # worst branch
        p = (g1 * g2.astype(np.float32)).astype(np.float16)
        outf = W.T @ p.astype(np.float32)
        out[sl] = _unpack_out(outf)
    return out


# revision 6
# speedup vs baseline: 1.2129x; 1.0537x over previous
"""Trainium2 Bass kernel for nn_CGPCoupler (sparse Clebsch-Gordan bilinear coupling).

Reference computation:
    out[:, ro] += x1[:, r1] * x2[:, r2] * cg        (nnz = 9856 sparse entries)

Structure exploited: the index triples come in 16-wide aligned runs, so the whole
op factors over 16-element "subslots" (40 of them in the 640-dim rep space):

    out_O  +=  c_t * (x1_A  (*)  x2_B)      for 616 subslot-triples t=(A,B,O,c)

with only D=308 distinct (A,B) products (provably minimal: every (l1,l2) family
couples to all its allowed lout, so the joint coupling tensor has full product
rank). Dataflow (per core, data parallel over the batch dim, 1024 rows/core,
fp16 datapath / fp32 PSUM):

    layout:  x2f[p = subslot*2 + ch_half (80 partitions), f = n*8 + ch_lo (8192)]
    host:    x1g = x1 replicated into product-row order (numpy fancy-index),
             streamed straight from HBM (no on-chip gather for side 1)
    1. G2 = SEL2^T @ x2f      (TensorE one-hot selection matmul -> PSUM)
    2. P  = x1g * G2          (VectorE 2x fp16 after ScalarE evacuates some
                               chunks; VectorE multiplies the rest straight
                               out of PSUM at 1x)
    3. out = W^T @ P          (TensorE, CG coeffs folded into constant fp16 W,
                               PSUM-accumulated over the 5 product-row chunks)

v5 pipeline (vs v3 baseline at 62.3us):
  - stationary reuse: gather/scatter loop chunks OUTER over 2-super blocks, so
    each SEL2/W chunk is loaded once per 4 matmuls (was 1:1) -> ~4x fewer
    LDWEIGHTS on the PE pipe.
  - exact 8-bank PSUM plan: 4 rotating gather banks [128,512]f32 + 2x2-bank
    output accumulators [80,1024]f32 (two supers in flight).
  - measured-cost-balanced evacuation: ScalarE evacuates NEVAC of 10 segs
    (0.61us each) + VectorE multiplies them in 2x fp16 (0.33us); VectorE
    multiplies the rest directly from PSUM at 1x (0.66us).
  - piecewise x2f loads (per-super 160KB) so the first gather starts ~0.6us in.
"""

import os
import sys
import types

import numpy as np


def _ensure_ntff_hook():
    """concourse's trace path imports antenv.axon_hooks, which this image's
    antenv lacks. Provide it (and register the real profiling hook when the
    axon boot module is available) so tracing works instead of crashing."""
    try:
        import antenv
    except ImportError:
        return
    if getattr(antenv, "axon_hooks", None) is not None:
        return
    try:
        from antenv import axon_hooks  # noqa: F401
        return
    except ImportError:
        pass
    mod = types.ModuleType("antenv.axon_hooks")
    state = {"hook": None}
    mod.set_axon_ntff_profile_hook = lambda h: state.__setitem__("hook", h)
    mod.get_axon_ntff_profile_hook = lambda: state["hook"]
    sys.modules["antenv.axon_hooks"] = mod
    antenv.axon_hooks = mod
    try:
        from trn_agent_boot.trn_boot import _ntff_profile_via_ctypes
        so = "/opt/axon/libaxon_pjrt.so"
        if os.path.exists(so):
            mod.set_axon_ntff_profile_hook(_ntff_profile_via_ctypes(so))
    except Exception:
        pass


_ensure_ntff_hook()

N = 8192
DIM = 640
NCORES = 8
NLOC = N // NCORES          # rows per core
NSUB = DIM // 16            # 40 subslots
P_IN = NSUB * 2             # 80 partitions: (subslot, ch-half)
CHH = 8                     # channels per half
FTOT = NLOC * CHH           # 8192 free elements per partition
FSUP = 1024                 # free-dim super-chunk (per DMA / out tile)
FSEG = 512                  # free-dim segment per matmul (one PSUM bank, fp32)
NSUP = FTOT // FSUP         # 8 supers
NSEG = FSUP // FSEG         # 2 segments per super
SBLK = 2                    # supers per stationary block

LAST_RESULTS = None         # BassKernelResults of the most recent run

_matrices_cache = {}
_program_cache = {}


def _build_matrices(cg, r1, r2, ro):
    """Derive subslot terms from the sparse index lists and build the constant
    SEL2/W matrices. Everything is validated with asserts."""
    key = (r1.tobytes(), r2.tobytes(), ro.tobytes(), cg.tobytes())
    hit = _matrices_cache.get(key)
    if hit is not None:
        return hit

    A = r1 // 16
    B = r2 // 16
    O = ro // 16
    j = r1 % 16
    assert (r2 % 16 == j).all() and (ro % 16 == j).all(), \
        "index triples are not 16-aligned runs"
    assert A.max() < NSUB and B.max() < NSUB and O.max() < NSUB

    terms = {}   # (A,B,O) -> [coeff, covered-bitmask]
    for a, b, o, jj, c in zip(A.tolist(), B.tolist(), O.tolist(),
                              j.tolist(), cg.tolist()):
        k = (a, b, o)
        e = terms.get(k)
        if e is None:
            terms[k] = [c, 1 << jj]
        else:
            assert e[0] == c, "coefficient varies within a 16-run"
            assert not (e[1] >> jj) & 1, "duplicate (A,B,O,j) entry"
            e[1] |= 1 << jj
    for k, (c, mask) in terms.items():
        assert mask == 0xFFFF, f"term {k} covers only mask {mask:#x}"

    products = sorted({(a, b) for (a, b, o) in terms})
    pidx = {ab: d for d, ab in enumerate(products)}
    D = len(products)
    D2 = 2 * D
    nchunks = (D2 + 127) // 128
    D2p = 128 * nchunks

    SEL2 = np.zeros((P_IN, D2p), np.float16)
    A2 = np.zeros(D2p, np.int64)   # product row -> source row in x1f layout
    W = np.zeros((D2p, P_IN), np.float16)
    for (a, b), d in pidx.items():
        for hh in (0, 1):
            SEL2[b * 2 + hh, 2 * d + hh] = 1.0
            A2[2 * d + hh] = a * 2 + hh
    for (a, b, o), (c, _) in terms.items():
        d = pidx[(a, b)]
        for hh in (0, 1):
            W[2 * d + hh, o * 2 + hh] = c

    # pack W row-chunks side by side: WPACK[:, c*P_IN:(c+1)*P_IN] = W[c*128:...]
    WPACK = np.zeros((128, nchunks * P_IN), np.float16)
    for c in range(nchunks):
        WPACK[:, c * P_IN:(c + 1) * P_IN] = W[c * 128:(c + 1) * 128, :]

    out = (A2, SEL2, WPACK, nchunks)
    _matrices_cache[key] = out
    return out


def _pack_x(x):
    """[NLOC, 640] -> [80, NLOC*8] fp16: row p = subslot*2 + half, col = n*8 + ch."""
    return np.ascontiguousarray(
        x.reshape(NLOC, NSUB, 2, CHH).transpose(1, 2, 0, 3).reshape(P_IN, FTOT),
        dtype=np.float16)


def _unpack_out(o):
    """[80, NLOC*8] -> [NLOC, 640]."""
    return o.reshape(NSUB, 2, NLOC, CHH).transpose(2, 0, 1, 3).reshape(NLOC, DIM)


# ---- tuning knobs -----------------------------------------------------------
NEVAC = 6      # of the 10 gather segments per super, how many ScalarE evacuates
               # (VectorE handles the rest straight from PSUM at 1x)


def _build_program(nchunks):
    import concourse.mybir as mybir
    import concourse.tile as tile
    from concourse import bacc
    from concourse.bass import ds, ts

    f32 = mybir.dt.float32
    f16 = mybir.dt.float16
    nc = bacc.Bacc("TRN2", target_bir_lowering=False)

    x1gd = nc.dram_tensor("x1g", [nchunks, 128, FTOT], f16, kind="ExternalInput")
    x2d = nc.dram_tensor("x2f", [P_IN, FTOT], f16, kind="ExternalInput")
    s2d = nc.dram_tensor("sel2", [P_IN, nchunks * 128], f16, kind="ExternalInput")
    wd = nc.dram_tensor("wmat", [128, nchunks * P_IN], f16, kind="ExternalInput")
    outd = nc.dram_tensor("outf", [P_IN, FTOT], f16, kind="ExternalOutput")

    NBLK = NSUP // SBLK

    with tile.TileContext(nc) as tc:
        with tc.tile_pool(name="const", bufs=1) as constp, \
             tc.tile_pool(name="x1io", bufs=3 * nchunks) as x1io, \
             tc.tile_pool(name="x2io", bufs=3) as x2io, \
             tc.tile_pool(name="gsb", bufs=14) as gsb, \
             tc.tile_pool(name="psb", bufs=3 * SBLK * NSEG * nchunks) as psb, \
             tc.tile_pool(name="og", bufs=4) as og, \
             tc.tile_pool(name="psg", bufs=5, space="PSUM") as psg, \
             tc.tile_pool(name="pso", bufs=3, space="PSUM") as pso:

            s2 = constp.tile([P_IN, nchunks * 128], f16, name="s2", tag="s2")
            nc.scalar.dma_start(out=s2, in_=s2d[:])
            w = constp.tile([128, nchunks * P_IN], f16, name="w", tag="w")
            nc.scalar.dma_start(out=w, in_=wd[:])

            for blk in range(NBLK):
                sups = [blk * SBLK + i for i in range(SBLK)]

                # ---- input DMAs for this block -----------------------------
                # x2 pieces ride the (otherwise idle) gpsimd SWDGE queue so
                # they never queue behind ScalarE evacuation work; x1g comes
                # in block-sized [128, 2048] transfers so the sync engine's
                # ~0.7us per-DMA issue cost stays well under the wire time.
                blk0 = blk * SBLK * FSUP
                x2b = x2io.tile([P_IN, SBLK * FSUP], f16, name="x2b", tag="x2b")
                nc.gpsimd.dma_start(out=x2b, in_=x2d[:, ds(blk0, SBLK * FSUP)])
                x2t = {s: x2b[:, ts(i, FSUP)] for i, s in enumerate(sups)}
                x1gb = {}
                for c in range(nchunks):
                    g = x1io.tile([128, SBLK * FSUP], f16, name="x1g", tag="x1g")
                    nc.sync.dma_start(
                        out=g, in_=x1gd[c, :, blk0:blk0 + SBLK * FSUP])
                    x1gb[c] = g
                x1gt = {(s, c): x1gb[c][:, ts(i, FSUP)]
                        for i, s in enumerate(sups) for c in range(nchunks)}

                # ---- gather + multiply ------------------------------------
                # s-outer so each super's product tiles complete ASAP for the
                # scatter. Block 0 evacuates everything: the direct-PSUM
                # multiply path would hold gather banks hostage while the
                # first x1g block (2.6 MB) is still on the wire.
                pt = {}
                evac_budget = 10 * SBLK if blk == 0 else NEVAC * SBLK
                ei = 0
                for s in sups:
                    for c in range(nchunks):
                        g2p = psg.tile([128, FSEG], f32, name="gp", tag="gp")
                        g2p2 = psg.tile([128, FSEG], f32, name="gp2", tag="gp")
                        segs = (g2p, g2p2)
                        for jseg in range(NSEG):
                            nc.tensor.matmul(
                                segs[jseg], s2[:, ts(c, 128)],
                                x2t[s][:, ts(jseg, FSEG)],
                                start=True, stop=True)
                        for jseg in range(NSEG):
                            p = psb.tile([128, FSEG], f16, name="pt", tag="pt")
                            x1seg = x1gt[s, c][:, ts(jseg, FSEG)]
                            if ei < evac_budget:
                                # ScalarE evacuates; VectorE multiplies 2x fp16
                                g2s = gsb.tile([128, FSEG], f16, name="g2s", tag="g2s")
                                nc.scalar.copy(out=g2s, in_=segs[jseg])
                                nc.vector.tensor_mul(p, x1seg, g2s)
                            else:
                                # VectorE multiplies straight from PSUM (1x)
                                nc.vector.tensor_mul(p, x1seg, segs[jseg])
                            ei += 1
                            pt[s, c, jseg] = p

                # ---- scatter: c-inner so each [80,512] accumulator lives
                # only ~2us in PSUM, then is cast out immediately ------------
                last = (blk == NBLK - 1)
                for i, s in enumerate(sups):
                    outt = og.tile([P_IN, FSUP], f16, name="outt", tag="outt")
                    for jseg in range(NSEG):
                        acc = pso.tile([P_IN, FSEG], f32, name="acc", tag="acc")
                        for c in range(nchunks):
                            nc.tensor.matmul(
                                acc, w[:, ts(c, P_IN)], pt[s, c, jseg],
                                start=(c == 0), stop=(c == nchunks - 1),
                                skip_group_check=True)
                        # alternate the cast between VectorE and ScalarE
                        if jseg % 2 == 0:
                            nc.vector.tensor_copy(out=outt[:, ts(jseg, FSEG)],
                                                  in_=acc)
                        else:
                            nc.scalar.copy(out=outt[:, ts(jseg, FSEG)],
                                           in_=acc)
                    ssl = ds(s * FSUP, FSUP)
                    if last and i == SBLK - 1:
                        # kernel tail: low-latency HWDGE path
                        nc.scalar.dma_start(out=outd[:, ssl], in_=outt)
                    else:
                        nc.gpsimd.dma_start(out=outd[:, ssl], in_=outt)
    nc.compile()
    return nc


def kernel(x1, x2, cg_tilde, repids_in1, repids_in2, repids_out, out_dim=DIM,
           **_ignored):
    global LAST_RESULTS
    import concourse.bass_utils as _bu
    from concourse.bass_utils import run_bass_kernel_spmd
    # the trace path uploads artifacts to S3, which this container can't reach
    if not getattr(_bu.upload_artifacts, "_local", False):
        _bu.upload_artifacts = lambda tmpdir: "local://" + tmpdir
        _bu.upload_artifacts._local = True

    x1 = np.ascontiguousarray(np.asarray(x1), dtype=np.float32)
    x2 = np.ascontiguousarray(np.asarray(x2), dtype=np.float32)
    cg = np.asarray(cg_tilde, dtype=np.float32)
    r1 = np.asarray(repids_in1, dtype=np.int64)
    r2 = np.asarray(repids_in2, dtype=np.int64)
    ro = np.asarray(repids_out, dtype=np.int64)
    out_dim = int(out_dim)
    assert x1.shape == (N, DIM) and x2.shape == (N, DIM) and out_dim == DIM

    A2, SEL2, WPACK, nchunks = _build_matrices(cg, r1, r2, ro)

    nc = _program_cache.get(nchunks)
    if nc is None:
        nc = _build_program(nchunks)
        _program_cache[nchunks] = nc

    in_maps = []
    for c in range(NCORES):
        sl = slice(c * NLOC, (c + 1) * NLOC)
        x1f = _pack_x(x1[sl])
        in_maps.append({
            "x1g": np.ascontiguousarray(
                x1f[A2].reshape(nchunks, 128, FTOT)),
            "x2f": _pack_x(x2[sl]),
            "sel2": SEL2,
            "wmat": WPACK,
        })

    res = run_bass_kernel_spmd(nc, in_maps, core_ids=list(range(NCORES)))
    LAST_RESULTS = res

    out = np.empty((N, DIM), np.float32)
    for c in range(NCORES):
        out[c * NLOC:(c + 1) * NLOC] = _unpack_out(
            np.asarray(res.results[c]["outf"], dtype=np.float32))
    return out


def _numpy_model(x1, x2, cg, r1, r2, ro):
    """Host-side model of the device dataflow (including fp16 quantization),
    for validating index logic and predicting the on-device error."""
    A2, SEL2, WPACK, nchunks = _build_matrices(cg, r1, r2, ro)
    W = np.zeros((128 * nchunks, P_IN), np.float32)
    for c in range(nchunks):
        W[c * 128:(c + 1) * 128, :] = WPACK[:, c * P_IN:(c + 1) * P_IN].astype(
            np.float32)
    out = np.empty_like(x1)
    for c in range(NCORES):
        sl = slice(c * NLOC, (c + 1) * NLOC)
        x1f = _pack_x(x1[sl])
        x2f = _pack_x(x2[sl]).astype(np.float32)
        g1 = x1f[A2].astype(np.float32)
        g2 = (SEL2.astype(np.float32).T @ x2f).astype(np.float16)  # worst branch
        p = (g1 * g2.astype(np.float32)).astype(np.float16)
        outf = W.T @ p.astype(np.float32)
        out[sl] = _unpack_out(outf)
    return out
